# revision 15
# baseline (speedup 1.0000x reference)
"""Trainium2 Bass kernel for nn_AblationAttention (GQA causal attention with
QK-RMSNorm), sharded over 8 NeuronCores.

Problem (hardcoded): B=2, S=2048, E=2048, H=16, KV=8, D=128, G=2.
  q = x@Wq.T+bq; k = x@Wk.T+bk; v = x@Wv.T+bv   (heads split on E)
  q,k: per-head RMSNorm over D (eps = f32 eps), weights qn_w/kn_w
  GQA: kv head c serves q heads 2c, 2c+1; causal softmax(q k / sqrt(D)) @ v
  out = attn_out @ Wo.T + bo

Sharding: core c owns kv head c and q heads {2c, 2c+1} (tensor parallel).
Each core computes its 2 heads' attention output (B,S,256) and a row-parallel
partial of the output projection (B,S,E); host sums the 8 partials (+bo).

Device layout is "transposed" (feature on partitions, tokens on free dim):
  qT/kT: (D, tokens); scoresT tile: (kt, qt) = kT_chunk.T @ qT  (PE matmul)
  softmax denominators replicated over partitions via ones-matrix matmuls,
  normalization via fast approximate reciprocal + multiply (DVE only)
  out_T = v_chunk.T @ expT accumulated over kt chunks
Matmuls run in float32r (1 cycle/row at free-dim>=256, ~1e-4 rel err).
Batch 1's projection is interleaved with batch 0's output projection to
keep the PE busy while x streams from HBM.
"""

import os
import sys

for _p in ("/opt/trn_rl_repo", "/root/.axon_site/_ro/trn_rl_repo"):
    if os.path.isdir(_p) and _p not in sys.path:
        sys.path.append(_p)

import numpy as np

import concourse.bass as bass
import concourse.tile as tile
from concourse import bacc, mybir
from concourse.bass_utils import run_bass_kernel_spmd
from concourse.masks import make_identity

B, S, E = 2, 2048, 2048
H, KV, D = 16, 8, 128
G = H // KV
BS = B * S
EPS = float(np.finfo(np.float32).eps)
NCORES = 8

TB = 512  # token block (psum free dim)
NTBB = S // TB  # 4 token blocks per batch
NKI = S // 128  # 16 key chunks per batch
NQI = S // TB  # 4 query blocks per batch
NET = E // 128  # 16 e-tiles
NEG = -1.0e30
LOOKAHEAD = 3

F32 = mybir.dt.float32
F32R = mybir.dt.float32r
AF = mybir.ActivationFunctionType


def build_nc():
    nc = bacc.Bacc()

    xT = nc.dram_tensor("xT", [E, BS], F32R, kind="ExternalInput")
    wqkvT = nc.dram_tensor("wqkvT", [E, 512], F32R, kind="ExternalInput")
    woT = nc.dram_tensor("woT", [2 * D, E], F32R, kind="ExternalInput")
    bqkv = nc.dram_tensor("bqkv", [128, 4], F32, kind="ExternalInput")
    wqk = nc.dram_tensor("wqk", [128, 1], F32, kind="ExternalInput")
    masks = nc.dram_tensor("masks", [128, 4, TB], F32, kind="ExternalInput")
    yT = nc.dram_tensor("yT", [E, BS], F32, kind="ExternalOutput")

    with tile.TileContext(nc) as tc:
        with (
            tc.tile_pool(name="consts", bufs=1) as consts,
            tc.tile_pool(name="acts", bufs=1) as acts,
            tc.tile_pool(name="xp", bufs=10) as xp,
            tc.tile_pool(name="vtp", bufs=2) as vtp,
            tc.tile_pool(name="sqp", bufs=3) as sqp,
            tc.tile_pool(name="expp", bufs=4) as expp,
            tc.tile_pool(name="osh", bufs=1) as osh,
            tc.tile_pool(name="yp", bufs=4) as ypool,
            tc.tile_pool(name="linvp", bufs=2) as linvp,
        ):
            # ---- resident constants / weights ----
            w_sb = consts.tile([128, NET, 512], F32R)
            wo_sb = consts.tile([128, 2, E], F32R)
            b_sb = consts.tile([128, 4], F32)
            nc.sync.dma_start(out=b_sb, in_=bqkv[:, :])
            wqk_sb = consts.tile([128, 1], F32)
            nc.sync.dma_start(out=wqk_sb, in_=wqk[:, :])
            mask_sb = consts.tile([128, 4, TB], F32)
            nc.sync.dma_start(out=mask_sb, in_=masks[:, :, :])
            ident = consts.tile([128, 128], F32)
            make_identity(nc, ident)
            ones_f32 = consts.tile([128, 128], F32)
            nc.vector.memset(ones_f32, 1.0)
            ones_red = consts.tile([128, 1], F32R)
            nc.vector.tensor_copy(ones_red, ones_f32[:, 0:1])
            ones_row = consts.tile([1, 128], F32R)
            nc.vector.tensor_copy(ones_row, ones_f32[0:1, :])
            ones_sq = consts.tile([128, 128], F32R)
            nc.vector.tensor_copy(ones_sq, ones_f32)
            eps_sb = consts.tile([128, 1], F32)
            nc.vector.memset(eps_sb, EPS)
            deps_sb = consts.tile([128, 1], F32)
            nc.vector.memset(deps_sb, float(D) * EPS)

            # ---- resident activations ----
            q_sb = acts.tile([128, G, BS], F32R)
            k_sb = acts.tile([128, BS], F32R)
            vn_sb = acts.tile([128, B * NKI, 128], F32R)
            beta_sb = acts.tile([128, B, NKI], F32)
            alpha_sb = acts.tile([1, S], F32R)
            sqms_sb = acts.tile([1, S], F32)

            def stage_weights_qkv(et):
                nc.sync.dma_start(
                    out=w_sb[:, et, :], in_=wqkvT[et * 128 : (et + 1) * 128, :]
                )

            def stage_weights_wo():
                for ct in range(2):
                    nc.sync.dma_start(
                        out=wo_sb[:, ct, :], in_=woT[ct * 128 : (ct + 1) * 128, :]
                    )

            def proj_tb(tb, ps1, psvt, stage_w=False):
                """Project one global token block (512 tokens) to q/k/v."""
                ps_acc = [
                    ps1.tile([128, TB], F32, tag=f"acc{j}", name=f"acc{j}")
                    for j in range(4)
                ]
                for et in range(NET):
                    if stage_w:
                        stage_weights_qkv(et)
                    x_t = xp.tile([128, TB], F32R, tag="xr", name="x_t")
                    nc.sync.dma_start(
                        out=x_t,
                        in_=xT[et * 128 : (et + 1) * 128, tb * TB : (tb + 1) * TB],
                    )
                    for j in range(4):
                        nc.tensor.matmul(
                            ps_acc[j],
                            w_sb[:, et, j * 128 : (j + 1) * 128],
                            x_t,
                            start=(et == 0),
                            stop=(et == NET - 1),
                        )
                tsl = slice(tb * TB, (tb + 1) * TB)
                for j in range(2):
                    nc.scalar.activation(
                        out=q_sb[:, j, tsl],
                        in_=ps_acc[j],
                        func=AF.Identity,
                        bias=b_sb[:, j : j + 1],
                        scale=1.0,
                    )
                nc.vector.tensor_scalar(
                    out=k_sb[:, tsl],
                    in0=ps_acc[2],
                    scalar1=b_sb[:, 2:3],
                    scalar2=None,
                    op0=mybir.AluOpType.add,
                )
                vt_tmp = vtp.tile([128, TB], F32, tag="vt", name="vt_tmp")
                nc.vector.tensor_scalar(
                    out=vt_tmp,
                    in0=ps_acc[3],
                    scalar1=b_sb[:, 3:4],
                    scalar2=None,
                    op0=mybir.AluOpType.add,
                )
                for t in range(4):
                    vt_ps = psvt.tile([128, 128], F32, tag="vtps", name="vt_ps")
                    nc.tensor.transpose(
                        vt_ps, vt_tmp[:, t * 128 : (t + 1) * 128], ident
                    )
                    nc.vector.tensor_copy(vn_sb[:, tb * 4 + t, :], vt_ps)

            def rms_b(b, ps2, ps2b):
                """RMS factors for batch b (uses the Sqrt ACT table set)."""
                for ki in range(NKI):
                    sl = slice(b * S + ki * 128, b * S + (ki + 1) * 128)
                    sqk = sqp.tile([128, 128], F32, tag="sqk", name="sqk")
                    nc.vector.tensor_mul(sqk, k_sb[:, sl], k_sb[:, sl])
                    bt_ps = ps2b.tile([128, 1], F32, tag="btp", name="bt_ps")
                    nc.tensor.matmul(
                        bt_ps, sqk, ones_f32[:, 0:1], start=True, stop=True
                    )
                    nc.scalar.activation(
                        out=beta_sb[:, b, ki : ki + 1],
                        in_=bt_ps,
                        func=AF.Sqrt,
                        bias=deps_sb,
                        scale=1.0,
                    )
                nc.vector.reciprocal(beta_sb[:, b, :], beta_sb[:, b, :])
                # fold qn_w * kn_w into k (valid: post-RMS per-d scale)
                nc.vector.tensor_scalar_mul(
                    k_sb[:, b * S : (b + 1) * S],
                    in0=k_sb[:, b * S : (b + 1) * S],
                    scalar1=wqk_sb,
                )
                for h in range(G):
                    qsl = q_sb[:, h, b * S : (b + 1) * S]
                    for t in range(NQI):
                        tsl = slice(t * TB, (t + 1) * TB)
                        sq = sqp.tile([128, TB], F32R, tag="sq", name="sq")
                        nc.vector.tensor_mul(sq, qsl[:, tsl], qsl[:, tsl])
                        sm_ps = ps2.tile([1, TB], F32, tag="smp", name="sm_ps")
                        nc.tensor.matmul(sm_ps, ones_red, sq, start=True, stop=True)
                        nc.scalar.activation(
                            out=sqms_sb[:, tsl],
                            in_=sm_ps,
                            func=AF.Sqrt,
                            bias=eps_sb[:1],
                            scale=1.0 / D,
                        )
                    with nc.allow_low_precision(reason="f32r rounding for PE"):
                        nc.vector.reciprocal(alpha_sb, sqms_sb)
                    for t in range(NQI):
                        tsl = slice(t * TB, (t + 1) * TB)
                        bc_ps = ps2.tile([128, TB], F32, tag="bcp", name="bc_ps")
                        nc.tensor.matmul(
                            bc_ps, ones_row, alpha_sb[:, tsl], start=True, stop=True
                        )
                        nc.vector.tensor_mul(qsl[:, tsl], qsl[:, tsl], bc_ps)

            def attn_b(b, out_b, ps3, ps3o, ps3l):
                """Causal attention for both local heads of batch b."""
                for h in range(G):
                    qsl = q_sb[:, h, b * S : (b + 1) * S]
                    for qi in range(NQI):
                        nki = 4 * qi + 4
                        qblk = qsl[:, qi * TB : (qi + 1) * TB]
                        o_ps = ps3o.tile([128, TB], F32, tag="op", name="o_ps")
                        l_ps = ps3l.tile([128, TB], F32, tag="lp", name="l_ps")

                        def scores(ki, sc_ps):
                            ksl = k_sb[:, b * S + ki * 128 : b * S + (ki + 1) * 128]
                            nc.tensor.matmul(sc_ps, ksl, qblk, start=True, stop=True)
                            if ki >= 4 * qi:
                                nc.vector.tensor_add(
                                    sc_ps, sc_ps, mask_sb[:, ki - 4 * qi, :]
                                )

                        sc_tiles = {}
                        for ki in range(min(LOOKAHEAD, nki)):
                            sc_tiles[ki] = ps3.tile(
                                [128, TB], F32, tag="sc", name="sc"
                            )
                            scores(ki, sc_tiles[ki])
                        for ki in range(nki):
                            if ki + LOOKAHEAD < nki:
                                sc_tiles[ki + LOOKAHEAD] = ps3.tile(
                                    [128, TB], F32, tag="sc", name="sc"
                                )
                                scores(ki + LOOKAHEAD, sc_tiles[ki + LOOKAHEAD])
                            e_sb = expp.tile([128, TB], F32R, tag="exp", name="e_sb")
                            nc.scalar.activation(
                                out=e_sb,
                                in_=sc_tiles.pop(ki),
                                func=AF.Exp,
                                scale=beta_sb[:, b, ki : ki + 1],
                            )
                            nc.tensor.matmul(
                                o_ps,
                                vn_sb[:, b * NKI + ki, :],
                                e_sb,
                                start=(ki == 0),
                                stop=(ki == nki - 1),
                            )
                            nc.tensor.matmul(
                                l_ps,
                                ones_sq,
                                e_sb,
                                start=(ki == 0),
                                stop=(ki == nki - 1),
                            )
                        linv = linvp.tile([128, TB], F32, tag="linv", name="linv")
                        nc.vector.reciprocal_approx_fast(linv, l_ps)
                        nc.vector.tensor_mul(
                            out_b[:, h, qi * TB : (qi + 1) * TB], linv, o_ps
                        )

            def outproj_t(b, t, out_b, psy):
                """Output projection partial for token block t of batch b."""
                for et in range(NET):
                    y_ps = psy.tile([128, TB], F32, tag="yp", name="y_ps")
                    for ct in range(2):
                        nc.tensor.matmul(
                            y_ps,
                            wo_sb[:, ct, et * 128 : (et + 1) * 128],
                            out_b[:, ct, t * TB : (t + 1) * TB],
                            start=(ct == 0),
                            stop=(ct == 1),
                        )
                    y_sb = ypool.tile([128, TB], F32, tag="y", name="y_sb")
                    if et % 2 == 0:
                        nc.scalar.copy(out=y_sb, in_=y_ps)
                    else:
                        nc.vector.tensor_copy(y_sb, y_ps)
                    nc.sync.dma_start(
                        out=yT[
                            et * 128 : (et + 1) * 128,
                            (b * NQI + t) * TB : (b * NQI + t + 1) * TB,
                        ],
                        in_=y_sb,
                    )

            # =================== schedule ===================
            # A: proj b0 (+ weight staging)
            with (
                tc.tile_pool(name="ps1", bufs=1, space="PSUM") as ps1,
                tc.tile_pool(name="psvt", bufs=2, space="PSUM") as psvt,
            ):
                for tb in range(NTBB):
                    proj_tb(tb, ps1, psvt, stage_w=(tb == 0))
                stage_weights_wo()
            # B: rms b0
            with (
                tc.tile_pool(name="ps2", bufs=2, space="PSUM") as ps2,
                tc.tile_pool(name="ps2b", bufs=2, space="PSUM") as ps2b,
            ):
                rms_b(0, ps2, ps2b)
            # C: attention b0
            out_b0 = osh.tile([128, G, S], F32R, tag="outsh", name="out_b0")
            with (
                tc.tile_pool(name="ps3", bufs=4, space="PSUM") as ps3,
                tc.tile_pool(name="ps3o", bufs=2, space="PSUM") as ps3o,
                tc.tile_pool(name="ps3l", bufs=2, space="PSUM") as ps3l,
            ):
                attn_b(0, out_b0, ps3, ps3o, ps3l)
            # D: interleave proj b1 with outproj b0 (PE work covers x DMA)
            with (
                tc.tile_pool(name="ps4", bufs=1, space="PSUM") as ps4,
                tc.tile_pool(name="ps4vt", bufs=2, space="PSUM") as ps4vt,
                tc.tile_pool(name="ps4y", bufs=2, space="PSUM") as ps4y,
            ):
                for tb in range(NTBB):
                    proj_tb(NTBB + tb, ps4, ps4vt)
                    outproj_t(0, tb, out_b0, ps4y)
            # E: rms b1
            with (
                tc.tile_pool(name="ps5", bufs=2, space="PSUM") as ps5,
                tc.tile_pool(name="ps5b", bufs=2, space="PSUM") as ps5b,
            ):
                rms_b(1, ps5, ps5b)
            # F: attention b1
            out_b1 = osh.tile([128, G, S], F32R, tag="outsh", name="out_b1")
            with (
                tc.tile_pool(name="ps6", bufs=4, space="PSUM") as ps6,
                tc.tile_pool(name="ps6o", bufs=2, space="PSUM") as ps6o,
                tc.tile_pool(name="ps6l", bufs=2, space="PSUM") as ps6l,
            ):
                attn_b(1, out_b1, ps6, ps6o, ps6l)
            # G: outproj b1
            with tc.tile_pool(name="ps7", bufs=2, space="PSUM") as ps7:
                for t in range(NQI):
                    outproj_t(1, t, out_b1, ps7)

    nc.compile()
    return nc


def _prep_inputs(x, Wq, bq, Wk, bk, Wv, bv, Wo, bo, qn_w, kn_w):
    """Shard the full inputs into the 8 per-core input maps."""
    x = np.asarray(x, np.float32)
    xT = np.ascontiguousarray(x.reshape(BS, E).T)  # (E, BS)

    kt = np.arange(128)[:, None]
    qt = np.arange(TB)[None, :]
    masks = np.stack(
        [np.where(qt >= kt + 128 * j, 0.0, NEG).astype(np.float32) for j in range(4)],
        axis=1,
    )
    masks = np.ascontiguousarray(masks)  # (128, 4, 512)

    wqk = np.ascontiguousarray(
        (np.asarray(qn_w, np.float32) * np.asarray(kn_w, np.float32)).reshape(128, 1)
    )

    in_maps = []
    for c in range(NCORES):
        qrows = slice(2 * c * D, (2 * c + 2) * D)
        kvrows = slice(c * D, (c + 1) * D)
        wcat = np.concatenate(
            [np.asarray(Wq, np.float32)[qrows], np.asarray(Wk, np.float32)[kvrows],
             np.asarray(Wv, np.float32)[kvrows]], axis=0
        )
        wqkvT = np.ascontiguousarray(wcat.T)  # (E, 512)
        woT = np.ascontiguousarray(np.asarray(Wo, np.float32)[:, qrows].T)  # (256, E)
        bcat = np.concatenate(
            [np.asarray(bq, np.float32)[qrows], np.asarray(bk, np.float32)[kvrows],
             np.asarray(bv, np.float32)[kvrows]]
        )
        bqkv = np.ascontiguousarray(bcat.reshape(4, 128).T)  # (128, 4)
        in_maps.append(
            {
                "xT": xT,
                "wqkvT": wqkvT,
                "woT": woT,
                "bqkv": bqkv,
                "wqk": wqk,
                "masks": masks,
            }
        )
    return in_maps


def _unshard(results, bo):
    acc = np.zeros((E, BS), np.float64)
    for r in results:
        acc += r["yT"].astype(np.float64)
    y = acc.T.reshape(B, S, E) + np.asarray(bo, np.float64)[None, None, :]
    return y.astype(np.float32)


_NC_CACHE = []


def _get_nc():
    if not _NC_CACHE:
        _NC_CACHE.append(build_nc())
    return _NC_CACHE[0]


def run(inputs, trace=False):
    nc = _get_nc()
    in_maps = _prep_inputs(**inputs)
    res = run_bass_kernel_spmd(nc, in_maps, core_ids=list(range(NCORES)), trace=trace)
    out = _unshard(res.results, inputs["bo"])
    return out, res


def kernel(**inputs) -> np.ndarray:
    out, _ = run(inputs, trace=False)
    return out


# revision 16
# speedup vs baseline: 1.1387x; 1.1387x over previous
"""Trainium2 Bass kernel for nn_AblationAttention (GQA causal attention with
QK-RMSNorm), sharded over 8 NeuronCores.

Problem (hardcoded): B=2, S=2048, E=2048, H=16, KV=8, D=128, G=2.
  q = x@Wq.T+bq; k = x@Wk.T+bk; v = x@Wv.T+bv   (heads split on E)
  q,k: per-head RMSNorm over D (eps = f32 eps), weights qn_w/kn_w
  GQA: kv head c serves q heads 2c, 2c+1; causal softmax(q k / sqrt(D)) @ v
  out = attn_out @ Wo.T + bo

Sharding: core c owns kv head c and q heads {2c, 2c+1} (tensor parallel).
Each core computes its 2 heads' attention output (B,S,256) and a row-parallel
partial of the output projection (B,S,E); host sums the 8 partials (+bo).

Device layout is "transposed" (feature on partitions, tokens on free dim):
  qT/kT: (D, tokens); scoresT tile: (kt, qt) = kT_chunk.T @ qT  (PE matmul)
  softmax denominators replicated over partitions via ones-matrix matmuls,
  normalization via fast approximate reciprocal + multiply (DVE only)
  out_T = v_chunk.T @ expT accumulated over kt chunks
Matmuls run in float32r (1 cycle/row at free-dim>=256, ~1e-4 rel err).
Batch 1's projection is interleaved with batch 0's output projection to
keep the PE busy while x streams from HBM.
"""

import os
import sys

for _p in ("/opt/trn_rl_repo", "/root/.axon_site/_ro/trn_rl_repo"):
    if os.path.isdir(_p) and _p not in sys.path:
        sys.path.append(_p)

import numpy as np

import concourse.bass as bass
import concourse.tile as tile
from concourse import bacc, mybir
from concourse.bass_utils import run_bass_kernel_spmd
from concourse.masks import make_identity

B, S, E = 2, 2048, 2048
H, KV, D = 16, 8, 128
G = H // KV
BS = B * S
EPS = float(np.finfo(np.float32).eps)
NCORES = 8

TB = 512  # token block (psum free dim)
NTBB = S // TB  # 4 token blocks per batch
NKI = S // 128  # 16 key chunks per batch
NQI = S // TB  # 4 query blocks per batch
NET = E // 128  # 16 e-tiles
NEG = -1.0e30
LOOKAHEAD = 3

F32 = mybir.dt.float32
F32R = mybir.dt.float32r
AF = mybir.ActivationFunctionType


def build_nc():
    nc = bacc.Bacc()

    xT = nc.dram_tensor("xT", [E, BS], F32, kind="ExternalInput")
    wqkvT = nc.dram_tensor("wqkvT", [E, 512], F32, kind="ExternalInput")
    woT = nc.dram_tensor("woT", [2 * D, E], F32, kind="ExternalInput")
    bqkv = nc.dram_tensor("bqkv", [128, 4], F32, kind="ExternalInput")
    wqk = nc.dram_tensor("wqk", [128, 1], F32, kind="ExternalInput")
    masks = nc.dram_tensor("masks", [128, 4, TB], F32, kind="ExternalInput")
    yT = nc.dram_tensor("yT", [E, BS], F32, kind="ExternalOutput")

    with tile.TileContext(nc) as tc:
        with (
            tc.tile_pool(name="consts", bufs=1) as consts,
            tc.tile_pool(name="acts", bufs=1) as acts,
            tc.tile_pool(name="xp", bufs=6) as xp,
            tc.tile_pool(name="vtp", bufs=2) as vtp,
            tc.tile_pool(name="sqp", bufs=3) as sqp,
            tc.tile_pool(name="expp", bufs=4) as expp,
            tc.tile_pool(name="osh", bufs=1) as osh,
            tc.tile_pool(name="yp", bufs=4) as ypool,
            tc.tile_pool(name="linvp", bufs=1) as linvp,
        ):
            # ---- resident constants / weights ----
            w_sb = consts.tile([128, NET, 512], F32R)
            wo_sb = consts.tile([128, 2, E], F32R)
            b_sb = consts.tile([128, 4], F32)
            nc.sync.dma_start(out=b_sb, in_=bqkv[:, :])
            wqk_sb = consts.tile([128, 1], F32)
            nc.sync.dma_start(out=wqk_sb, in_=wqk[:, :])
            mask_sb = consts.tile([128, 4, TB], F32)
            nc.sync.dma_start(out=mask_sb, in_=masks[:, :, :])
            ident = consts.tile([128, 128], F32)
            make_identity(nc, ident)
            ones_f32 = consts.tile([128, 128], F32)
            nc.vector.memset(ones_f32, 1.0)
            ones_red = consts.tile([128, 1], F32R)
            nc.vector.tensor_copy(ones_red, ones_f32[:, 0:1])
            ones_row = consts.tile([1, 128], F32R)
            nc.vector.tensor_copy(ones_row, ones_f32[0:1, :])
            ones_sq = consts.tile([128, 128], F32R)
            nc.vector.tensor_copy(ones_sq, ones_f32)
            eps_sb = consts.tile([128, 1], F32)
            nc.vector.memset(eps_sb, EPS)
            deps_sb = consts.tile([128, 1], F32)
            nc.vector.memset(deps_sb, float(D) * EPS)

            # ---- resident activations ----
            q_sb = acts.tile([128, G, BS], F32R)
            k_sb = acts.tile([128, BS], F32R)
            vn_sb = acts.tile([128, B * NKI, 128], F32R)
            beta_sb = acts.tile([128, B, NKI], F32)
            alpha_sb = acts.tile([1, S], F32R)
            sqms_sb = acts.tile([1, S], F32)

            def stage_weights_qkv(et):
                wst = xp.tile([128, 512], F32, tag="x", name="wst")
                nc.sync.dma_start(out=wst, in_=wqkvT[et * 128 : (et + 1) * 128, :])
                if et % 2 == 0:
                    nc.vector.tensor_copy(w_sb[:, et, :], wst)
                else:
                    nc.scalar.copy(out=w_sb[:, et, :], in_=wst)

            def stage_weights_wo():
                for ct in range(2):
                    for es in range(E // 512):
                        wst = xp.tile([128, 512], F32, tag="x", name="wst")
                        nc.sync.dma_start(
                            out=wst,
                            in_=woT[
                                ct * 128 : (ct + 1) * 128, es * 512 : (es + 1) * 512
                            ],
                        )
                        dst = wo_sb[:, ct, es * 512 : (es + 1) * 512]
                        if es % 2 == 0:
                            nc.vector.tensor_copy(dst, wst)
                        else:
                            nc.scalar.copy(out=dst, in_=wst)

            def proj_tb(tb, ps1, psvt, stage_w=False):
                """Project one global token block (512 tokens) to q/k/v."""
                ps_acc = [
                    ps1.tile([128, TB], F32, tag=f"acc{j}", name=f"acc{j}")
                    for j in range(4)
                ]
                for et in range(NET):
                    if stage_w:
                        stage_weights_qkv(et)
                    x_s = xp.tile([128, TB], F32, tag="x", name="x_s")
                    nc.sync.dma_start(
                        out=x_s,
                        in_=xT[et * 128 : (et + 1) * 128, tb * TB : (tb + 1) * TB],
                    )
                    x_t = xp.tile([128, TB], F32R, tag="xr", name="x_t")
                    if et % 2 == 0:
                        nc.vector.tensor_copy(x_t, x_s)
                    else:
                        nc.scalar.copy(out=x_t, in_=x_s)
                    for j in range(4):
                        nc.tensor.matmul(
                            ps_acc[j],
                            w_sb[:, et, j * 128 : (j + 1) * 128],
                            x_t,
                            start=(et == 0),
                            stop=(et == NET - 1),
                        )
                tsl = slice(tb * TB, (tb + 1) * TB)
                for j in range(2):
                    nc.scalar.activation(
                        out=q_sb[:, j, tsl],
                        in_=ps_acc[j],
                        func=AF.Identity,
                        bias=b_sb[:, j : j + 1],
                        scale=1.0,
                    )
                nc.vector.tensor_scalar(
                    out=k_sb[:, tsl],
                    in0=ps_acc[2],
                    scalar1=b_sb[:, 2:3],
                    scalar2=None,
                    op0=mybir.AluOpType.add,
                )
                vt_tmp = vtp.tile([128, TB], F32, tag="vt", name="vt_tmp")
                nc.vector.tensor_scalar(
                    out=vt_tmp,
                    in0=ps_acc[3],
                    scalar1=b_sb[:, 3:4],
                    scalar2=None,
                    op0=mybir.AluOpType.add,
                )
                for t in range(4):
                    vt_ps = psvt.tile([128, 128], F32, tag="vtps", name="vt_ps")
                    nc.tensor.transpose(
                        vt_ps, vt_tmp[:, t * 128 : (t + 1) * 128], ident
                    )
                    nc.vector.tensor_copy(vn_sb[:, tb * 4 + t, :], vt_ps)

            def rms_b(b, ps2, ps2b):
                """RMS factors for batch b (uses the Sqrt ACT table set)."""
                for ki in range(NKI):
                    sl = slice(b * S + ki * 128, b * S + (ki + 1) * 128)
                    sqk = sqp.tile([128, 128], F32, tag="sqk", name="sqk")
                    nc.vector.tensor_mul(sqk, k_sb[:, sl], k_sb[:, sl])
                    bt_ps = ps2b.tile([128, 1], F32, tag="btp", name="bt_ps")
                    nc.tensor.matmul(
                        bt_ps, sqk, ones_f32[:, 0:1], start=True, stop=True
                    )
                    nc.scalar.activation(
                        out=beta_sb[:, b, ki : ki + 1],
                        in_=bt_ps,
                        func=AF.Sqrt,
                        bias=deps_sb,
                        scale=1.0,
                    )
                nc.vector.reciprocal(beta_sb[:, b, :], beta_sb[:, b, :])
                # fold qn_w * kn_w into k (valid: post-RMS per-d scale)
                nc.vector.tensor_scalar_mul(
                    k_sb[:, b * S : (b + 1) * S],
                    in0=k_sb[:, b * S : (b + 1) * S],
                    scalar1=wqk_sb,
                )
                for h in range(G):
                    qsl = q_sb[:, h, b * S : (b + 1) * S]
                    for t in range(NQI):
                        tsl = slice(t * TB, (t + 1) * TB)
                        sq = sqp.tile([128, TB], F32R, tag="sq", name="sq")
                        nc.vector.tensor_mul(sq, qsl[:, tsl], qsl[:, tsl])
                        sm_ps = ps2.tile([1, TB], F32, tag="smp", name="sm_ps")
                        nc.tensor.matmul(sm_ps, ones_red, sq, start=True, stop=True)
                        nc.scalar.activation(
                            out=sqms_sb[:, tsl],
                            in_=sm_ps,
                            func=AF.Sqrt,
                            bias=eps_sb[:1],
                            scale=1.0 / D,
                        )
                    with nc.allow_low_precision(reason="f32r rounding for PE"):
                        nc.vector.reciprocal(alpha_sb, sqms_sb)
                    for t in range(NQI):
                        tsl = slice(t * TB, (t + 1) * TB)
                        bc_ps = ps2.tile([128, TB], F32, tag="bcp", name="bc_ps")
                        nc.tensor.matmul(
                            bc_ps, ones_row, alpha_sb[:, tsl], start=True, stop=True
                        )
                        nc.vector.tensor_mul(qsl[:, tsl], qsl[:, tsl], bc_ps)

            def attn_b(b, out_b, ps3, ps3o, ps3l):
                """Causal attention for both local heads of batch b."""
                for h in range(G):
                    qsl = q_sb[:, h, b * S : (b + 1) * S]
                    for qi in range(NQI):
                        nki = 4 * qi + 4
                        qblk = qsl[:, qi * TB : (qi + 1) * TB]
                        o_ps = ps3o.tile([128, TB], F32, tag="op", name="o_ps")
                        l_ps = ps3l.tile([128, TB], F32, tag="lp", name="l_ps")

                        def scores(ki, sc_ps):
                            ksl = k_sb[:, b * S + ki * 128 : b * S + (ki + 1) * 128]
                            nc.tensor.matmul(sc_ps, ksl, qblk, start=True, stop=True)
                            if ki >= 4 * qi:
                                nc.vector.tensor_add(
                                    sc_ps, sc_ps, mask_sb[:, ki - 4 * qi, :]
                                )

                        sc_tiles = {}
                        for ki in range(min(LOOKAHEAD, nki)):
                            sc_tiles[ki] = ps3.tile(
                                [128, TB], F32, tag="sc", name="sc"
                            )
                            scores(ki, sc_tiles[ki])
                        for ki in range(nki):
                            if ki + LOOKAHEAD < nki:
                                sc_tiles[ki + LOOKAHEAD] = ps3.tile(
                                    [128, TB], F32, tag="sc", name="sc"
                                )
                                scores(ki + LOOKAHEAD, sc_tiles[ki + LOOKAHEAD])
                            e_sb = expp.tile([128, TB], F32R, tag="exp", name="e_sb")
                            nc.scalar.activation(
                                out=e_sb,
                                in_=sc_tiles.pop(ki),
                                func=AF.Exp,
                                scale=beta_sb[:, b, ki : ki + 1],
                            )
                            nc.tensor.matmul(
                                o_ps,
                                vn_sb[:, b * NKI + ki, :],
                                e_sb,
                                start=(ki == 0),
                                stop=(ki == nki - 1),
                            )
                            nc.tensor.matmul(
                                l_ps,
                                ones_sq,
                                e_sb,
                                start=(ki == 0),
                                stop=(ki == nki - 1),
                            )
                        linv = linvp.tile([128, TB], F32, tag="linv", name="linv")
                        nc.vector.reciprocal_approx_fast(linv, l_ps)
                        nc.vector.tensor_mul(
                            out_b[:, h, qi * TB : (qi + 1) * TB], linv, o_ps
                        )

            def outproj_t(b, t, out_b, psy):
                """Output projection partial for token block t of batch b."""
                for et in range(NET):
                    y_ps = psy.tile([128, TB], F32, tag="yp", name="y_ps")
                    for ct in range(2):
                        nc.tensor.matmul(
                            y_ps,
                            wo_sb[:, ct, et * 128 : (et + 1) * 128],
                            out_b[:, ct, t * TB : (t + 1) * TB],
                            start=(ct == 0),
                            stop=(ct == 1),
                        )
                    y_sb = ypool.tile([128, TB], F32, tag="y", name="y_sb")
                    if et % 2 == 0:
                        nc.scalar.copy(out=y_sb, in_=y_ps)
                    else:
                        nc.vector.tensor_copy(y_sb, y_ps)
                    nc.sync.dma_start(
                        out=yT[
                            et * 128 : (et + 1) * 128,
                            (b * NQI + t) * TB : (b * NQI + t + 1) * TB,
                        ],
                        in_=y_sb,
                    )

            # =================== schedule ===================
            # A: proj b0 (+ weight staging)
            with (
                tc.tile_pool(name="ps1", bufs=1, space="PSUM") as ps1,
                tc.tile_pool(name="psvt", bufs=2, space="PSUM") as psvt,
            ):
                for tb in range(NTBB):
                    proj_tb(tb, ps1, psvt, stage_w=(tb == 0))
                stage_weights_wo()
            # B: rms b0
            with (
                tc.tile_pool(name="ps2", bufs=2, space="PSUM") as ps2,
                tc.tile_pool(name="ps2b", bufs=2, space="PSUM") as ps2b,
            ):
                rms_b(0, ps2, ps2b)
            # C: attention b0
            out_b0 = osh.tile([128, G, S], F32R, tag="outsh", name="out_b0")
            with (
                tc.tile_pool(name="ps3", bufs=4, space="PSUM") as ps3,
                tc.tile_pool(name="ps3o", bufs=2, space="PSUM") as ps3o,
                tc.tile_pool(name="ps3l", bufs=2, space="PSUM") as ps3l,
            ):
                attn_b(0, out_b0, ps3, ps3o, ps3l)
            # D: interleave proj b1 with outproj b0 (PE work covers x DMA)
            with (
                tc.tile_pool(name="ps4", bufs=1, space="PSUM") as ps4,
                tc.tile_pool(name="ps4vt", bufs=2, space="PSUM") as ps4vt,
                tc.tile_pool(name="ps4y", bufs=2, space="PSUM") as ps4y,
            ):
                for tb in range(NTBB):
                    proj_tb(NTBB + tb, ps4, ps4vt)
                    outproj_t(0, tb, out_b0, ps4y)
            # E: rms b1
            with (
                tc.tile_pool(name="ps5", bufs=2, space="PSUM") as ps5,
                tc.tile_pool(name="ps5b", bufs=2, space="PSUM") as ps5b,
            ):
                rms_b(1, ps5, ps5b)
            # F: attention b1
            out_b1 = osh.tile([128, G, S], F32R, tag="outsh", name="out_b1")
            with (
                tc.tile_pool(name="ps6", bufs=4, space="PSUM") as ps6,
                tc.tile_pool(name="ps6o", bufs=2, space="PSUM") as ps6o,
                tc.tile_pool(name="ps6l", bufs=2, space="PSUM") as ps6l,
            ):
                attn_b(1, out_b1, ps6, ps6o, ps6l)
            # G: outproj b1
            with tc.tile_pool(name="ps7", bufs=2, space="PSUM") as ps7:
                for t in range(NQI):
                    outproj_t(1, t, out_b1, ps7)

    nc.compile()
    return nc


def _prep_inputs(x, Wq, bq, Wk, bk, Wv, bv, Wo, bo, qn_w, kn_w):
    """Shard the full inputs into the 8 per-core input maps."""
    x = np.asarray(x, np.float32)
    xT = np.ascontiguousarray(x.reshape(BS, E).T)  # (E, BS)

    kt = np.arange(128)[:, None]
    qt = np.arange(TB)[None, :]
    masks = np.stack(
        [np.where(qt >= kt + 128 * j, 0.0, NEG).astype(np.float32) for j in range(4)],
        axis=1,
    )
    masks = np.ascontiguousarray(masks)  # (128, 4, 512)

    wqk = np.ascontiguousarray(
        (np.asarray(qn_w, np.float32) * np.asarray(kn_w, np.float32)).reshape(128, 1)
    )

    in_maps = []
    for c in range(NCORES):
        qrows = slice(2 * c * D, (2 * c + 2) * D)
        kvrows = slice(c * D, (c + 1) * D)
        wcat = np.concatenate(
            [np.asarray(Wq, np.float32)[qrows], np.asarray(Wk, np.float32)[kvrows],
             np.asarray(Wv, np.float32)[kvrows]], axis=0
        )
        wqkvT = np.ascontiguousarray(wcat.T)  # (E, 512)
        woT = np.ascontiguousarray(np.asarray(Wo, np.float32)[:, qrows].T)  # (256, E)
        bcat = np.concatenate(
            [np.asarray(bq, np.float32)[qrows], np.asarray(bk, np.float32)[kvrows],
             np.asarray(bv, np.float32)[kvrows]]
        )
        bqkv = np.ascontiguousarray(bcat.reshape(4, 128).T)  # (128, 4)
        in_maps.append(
            {
                "xT": xT,
                "wqkvT": wqkvT,
                "woT": woT,
                "bqkv": bqkv,
                "wqk": wqk,
                "masks": masks,
            }
        )
    return in_maps


def _unshard(results, bo):
    acc = np.zeros((E, BS), np.float64)
    for r in results:
        acc += r["yT"].astype(np.float64)
    y = acc.T.reshape(B, S, E) + np.asarray(bo, np.float64)[None, None, :]
    return y.astype(np.float32)


_NC_CACHE = []


def _get_nc():
    if not _NC_CACHE:
        _NC_CACHE.append(build_nc())
    return _NC_CACHE[0]


def run(inputs, trace=False):
    nc = _get_nc()
    in_maps = _prep_inputs(**inputs)
    res = run_bass_kernel_spmd(nc, in_maps, core_ids=list(range(NCORES)), trace=trace)
    out = _unshard(res.results, inputs["bo"])
    return out, res


def kernel(**inputs) -> np.ndarray:
    out, _ = run(inputs, trace=False)
    return out
